# revision 24
# baseline (speedup 1.0000x reference)
"""CrossMambaFusion Trainium2 kernel — 8-core SPMD via bass/Tile.

Sharding (hardcoded for B=2, C=256, H=W=64, Di=512, N=16, R=32, K=4):
  core c -> batch b = c//4, d_inner slice q = c%4 (128 channels).
  On-device layout is feature-major [features, tokens]; (B,C,H,W) inputs
  reshape to (C, L=4096) with no host transpose.

  SPMD trick 1: xm/u channel order is permuted per core (own d-slice first)
  via host-permuted weights, so "my slice" is always k-tile 0.
  SPMD trick 2: 8-rank AllToAll with duplicated quarter-shards; the m_out
  weight rows belonging to the other batch group are zeroed per core, so the
  K=1024 contraction drops cross-batch contributions (static offsets, SPMD).

v5 structure — the front and the selective scan are software-pipelined per
L-quarter: while the DVE scans quarter Q, the PE/ACT run the projection
front of quarter Q+1 ahead of it (in-order engine queues + Tile semaphores).
  - per quarter: phase1 (dec/enc proj+gate), in_proj, causal conv (DVE STT),
    x_proj, dt softplus (batched Exp/Ln tables), then 16-state scan with
    cross-quarter h carries; ysum accumulates on the PE via 4096*I matmuls
    into PSUM (fp8 pre-scale for free)
  - y ships through the 8-rank AllToAll as fp8e4m3; m_out runs fp8
  - the decoder-gate path fills the AllToAll latency bubble
"""
import numpy as np
import ml_dtypes

bf16 = ml_dtypes.bfloat16
f8e4 = ml_dtypes.float8_e4m3

B, C, Hh, Ww = 2, 256, 64, 64
L = Hh * Ww
Di, N, R, KC = 512, 16, 32, 4
DQ = 128
LQ = L // 4
NCORES = 8
LH = L // 2

YSC = 4096.0   # y fp8 pre-scale (via the eye accumulation matrix)
WSC = 256.0    # m_out weight fp8 pre-scale

_cache = {}


def _build():
    import concourse.bass as bass
    import concourse.mybir as mybir
    import concourse.tile as tile
    from concourse import bacc

    fp32 = mybir.dt.float32
    bfl = mybir.dt.bfloat16
    f8 = mybir.dt.float8e4
    AF = mybir.ActivationFunctionType
    OP = mybir.AluOpType
    ts = bass.ts

    nc = bacc.Bacc("TRN2", target_bir_lowering=False, num_devices=NCORES)

    def din(name, shape, dt=fp32):
        return nc.declare_dram_parameter(name, list(shape), dt, isOutput=False)

    dec_bf = din("dec_bf", (C, L), bfl)
    enc_bf = din("enc_bf", (C, L), bfl)
    dec_f32q = din("dec_f32q", (C, LQ), fp32)
    w_dec_x = din("w_dec_x", (C, Di), bfl)
    w_dec_g = din("w_dec_g", (C, Di), bfl)
    b_dec_x = din("b_dec_x", (Di, 1))
    b_dec_g = din("b_dec_g", (Di, 1))
    w_enc = din("w_enc", (C, Di), bfl)
    b_enc = din("b_enc", (Di, 1))
    w_in_x = din("w_in_x", (Di, Di), bfl)      # columns permuted (own slice first)
    b_in_x = din("b_in_x", (Di, 1))            # permuted
    w_in_z = din("w_in_z", (Di, DQ), bfl)
    b_in_z = din("b_in_z", (DQ, 1))
    conv_w4 = din("conv_w4", (Di, KC))         # permuted rows
    conv_b = din("conv_b", (Di, 1))            # permuted
    w_xp = din("w_xp", (Di, 2 * R), bfl)       # permuted rows
    w_dt = din("w_dt", (R, DQ), bfl)
    b_dt = din("b_dt", (DQ, 1))
    a_sl = din("a_sl", (DQ, N))
    d_col = din("d_col", (DQ, 1))
    w_mo8 = din("w_mo8", (2 * Di, Di), f8)     # rows of other batch group zeroed; x256
    b_mo = din("b_mo", (Di, 1))
    w_out = din("w_out", (Di, C), bfl)
    b_out = din("b_out", (C, 1))
    g_col = din("g_col", (C, 1))
    bln_col = din("bln_col", (C, 1))
    eye4k = din("eye4k", (128, 128), bfl)      # 4096 * I

    res_out = nc.declare_dram_parameter("res", [C, LQ], fp32, isOutput=True)

    LC = 512
    NL = L // LC
    NQ = 2          # pipeline stages (L halves)
    SQ = L // NQ    # tokens per stage
    CPQ = NL // NQ  # 512-col chunks per stage

    with tile.TileContext(nc) as tc:
        import contextlib
        with contextlib.ExitStack() as stack:
            wpool = stack.enter_context(tc.tile_pool(name="weights", bufs=1))
            cpool = stack.enter_context(tc.tile_pool(name="consts", bufs=1))
            dpool = stack.enter_context(tc.tile_pool(name="drambuf", bufs=1, space="DRAM"))

            bc_rows = dpool.tile([2 * N, L], bfl)     # B rows then C rows
            ln_rows = dpool.tile([2, LQ], fp32)       # mu, rstd
            a2a_in = dpool.tile([2 * Di, LQ], f8)
            a2a_out = dpool.tile([2 * Di, LQ], f8)
            a2a_warm_in = dpool.tile([8, 16], f8)
            a2a_warm_out = dpool.tile([8, 16], f8)

            def wload(ap, kt, m, name, dt=bfl):
                t = wpool.tile([128, kt, m], dt, tag=name, name=name)
                nc.sync.dma_start(out=t[:], in_=ap.ap().rearrange("(t k) m -> k t m", k=128))
                return t

            sw_dec_x = wload(w_dec_x, 2, Di, "w_dec_x")
            sw_enc = wload(w_enc, 2, Di, "w_enc")

            def cload(ap, nt, name, cols=1):
                if nt == 1:
                    t = cpool.tile([128, cols], fp32, tag=name, name=name)
                    nc.sync.dma_start(out=t[:], in_=ap.ap())
                else:
                    t = cpool.tile([128, nt, cols], fp32, tag=name, name=name)
                    nc.sync.dma_start(out=t[:], in_=ap.ap().rearrange("(t k) o -> k t o", k=128))
                return t

            sb_dec_x = cload(b_dec_x, 4, "b_dec_x")
            sb_dec_g = cload(b_dec_g, 4, "b_dec_g")
            sb_enc = cload(b_enc, 4, "b_enc")
            sb_in_x = cload(b_in_x, 4, "b_in_x")
            sb_in_z = cload(b_in_z, 1, "b_in_z")
            s_convw = cload(conv_w4, 4, "conv_w4", cols=KC)
            s_convb = cload(conv_b, 4, "conv_b")
            sb_dt = cload(b_dt, 1, "b_dt")
            s_a = cload(a_sl, 1, "a_sl", cols=N)
            s_d = cload(d_col, 1, "d_col")
            sb_mo = cload(b_mo, 4, "b_mo")
            sb_out = cload(b_out, 2, "b_out")
            s_g = cload(g_col, 2, "g_col")
            s_bln = cload(bln_col, 2, "bln_col")

            # persistent tiles
            spool0 = stack.enter_context(tc.tile_pool(name="scanin", bufs=1))
            s_dt = spool0.tile([128, L], bfl)
            s_dtu = spool0.tile([128, L], bfl)
            s_uD = spool0.tile([128, L], bfl)
            s_siluz = spool0.tile([128, L], bfl)
            s_y8 = spool0.tile([128, L], f8)
            s_carry = spool0.tile([128, N], bfl)      # per-n h carries across quarters
            s_ab = spool0.tile([128, L // 2], bfl)    # softplus staging (per stage)
            s_ex = spool0.tile([128, L // 2], bfl)

            sw_in_x = wload(w_in_x, 4, Di, "w_in_x")
            sw_in_z = wload(w_in_z, 4, DQ, "w_in_z")
            sw_xp = wload(w_xp, 4, 2 * R, "w_xp")
            sw_dt = wpool.tile([R, DQ], bfl)
            nc.sync.dma_start(out=sw_dt[:], in_=w_dt.ap())
            s_eye = wpool.tile([128, 128], bfl)
            nc.sync.dma_start(out=s_eye[:], in_=eye4k.ap())

            bca = bc_rows[0:1, 0:1]

            # ---- pipelined front + scan, one L-quarter at a time ----
            with tc.tile_pool(name="ph12", bufs=1) as ppool, \
                 tc.tile_pool(name="ph1c", bufs=3) as f1c, \
                 tc.tile_pool(name="ph2c", bufs=2) as m2c, \
                 tc.tile_pool(name="cmb", bufs=2) as cmbp, \
                 tc.tile_pool(name="scanh", bufs=3) as spool_h, \
                 tc.tile_pool(name="scanab", bufs=2) as spool_ab, \
                 tc.tile_pool(name="scanbc", bufs=2) as spool_bc, \
                 tc.tile_pool(name="ps1", bufs=1, space="PSUM") as ps1, \
                 tc.tile_pool(name="ps2x", bufs=1, space="PSUM") as ps2x, \
                 tc.tile_pool(name="ps2", bufs=1, space="PSUM") as ps2, \
                 tc.tile_pool(name="psy", bufs=1, space="PSUM") as psy:
                s_dec = ppool.tile([128, 2, L], bfl)
                s_enc = ppool.tile([128, 2, L], bfl)
                s_u = ppool.tile([128, 4, L], bfl)
                dec_r = dec_bf.ap().rearrange("(t k) l -> k t l", k=128)
                enc_r = enc_bf.ap().rearrange("(t k) l -> k t l", k=128)
                for pc in range(4):
                    pl = ts(pc, LQ)
                    nc.sync.dma_start(out=s_dec[:, :, pl], in_=dec_r[:, :, pl])
                    nc.sync.dma_start(out=s_enc[:, :, pl], in_=enc_r[:, :, pl])
                prev_xm = [None]

                def emit_ph1(lc):
                    ls = ts(lc, LC)
                    s_comb = cmbp.tile([128, 4, LC], bfl, tag="comb", name="comb")
                    for m in range(4):
                        ps_dx = ps1.tile([128, LC], fp32, tag="ps_dx", name="ps_dx")
                        ps_ep = ps1.tile([128, LC], fp32, tag="ps_ep", name="ps_ep")
                        for t in range(2):
                            nc.tensor.matmul(ps_dx[:], sw_dec_x[:, t, ts(m, 128)],
                                             s_dec[:, t, ls], start=(t == 0), stop=(t == 1))
                        for t in range(2):
                            nc.tensor.matmul(ps_ep[:], sw_enc[:, t, ts(m, 128)],
                                             s_enc[:, t, ls], start=(t == 0), stop=(t == 1))
                        sg = f1c.tile([128, LC], bfl, tag="sg", name="sg")
                        nc.scalar.activation(sg[:], ps_ep[:], AF.Sigmoid,
                                             bias=sb_enc[:, m, :])
                        tm = f1c.tile([128, LC], bfl, tag="tm", name="tm")
                        nc.vector.scalar_tensor_tensor(tm[:], ps_dx[:],
                                                       sb_dec_x[:, m, :], sg[:],
                                                       OP.add, OP.mult)
                        nc.vector.scalar_tensor_tensor(s_comb[:, m, :], ps_ep[:],
                                                       sb_enc[:, m, :], tm[:],
                                                       OP.add, OP.add)
                    return s_comb

                def emit_ph2(lc, s_comb, Q):
                    ls = ts(lc, LC)
                    s_xm = cmbp.tile([128, 4, 3 + LC], bfl, tag="xm", name="s_xm")
                    if prev_xm[0] is None:
                        nc.vector.memset(s_xm[:, :, 0:3], 0.0)
                    else:
                        nc.vector.tensor_copy(s_xm[:, :, 0:3],
                                              prev_xm[0][:, :, LC:LC + 3])
                    for m in range(4):
                        ps_xm = ps2x.tile([128, LC], fp32, tag="ps_xm", name="ps_xm")
                        for t in range(4):
                            nc.tensor.matmul(ps_xm[:], sw_in_x[:, t, ts(m, 128)],
                                             s_comb[:, t, :], start=(t == 0), stop=(t == 3))
                        nc.scalar.activation(s_xm[:, m, 3:3 + LC],
                                             ps_xm[:], AF.Identity, bias=sb_in_x[:, m, :])
                    prev_xm[0] = s_xm
                    ps_z = ps2.tile([128, LC], fp32, tag="zxd", name="ps_z")
                    for t in range(4):
                        nc.tensor.matmul(ps_z[:], sw_in_z[:, t, :], s_comb[:, t, :],
                                         start=(t == 0), stop=(t == 3))
                    nc.scalar.activation(s_siluz[:, ls], ps_z[:], AF.Silu,
                                         bias=sb_in_z[:, 0:1])
                    # causal depthwise conv on DVE (STT chains)
                    for m in range(4):
                        acc = m2c.tile([128, LC], fp32, tag=f"acc{m % 2}", name="acc")
                        nc.vector.tensor_scalar(acc[:], s_xm[:, m, 0:LC],
                                                s_convw[:, m, 0:1], None, OP.mult)
                        for k in range(1, KC):
                            nc.vector.scalar_tensor_tensor(
                                acc[:], s_xm[:, m, k:k + LC],
                                s_convw[:, m, k:k + 1], acc[:], OP.mult, OP.add)
                        nc.scalar.activation(s_u[:, m, ls], acc[:], AF.Silu,
                                             bias=s_convb[:, m, :])
                    # x_proj + B/C spill + dt-linear for this chunk
                    ps_xd = ps2.tile([128, LC], fp32, tag="zxd", name="ps_xd")
                    for t in range(4):
                        nc.tensor.matmul(ps_xd[0:64, :], sw_xp[:, t, :], s_u[:, t, ls],
                                         start=(t == 0), stop=(t == 3))
                    dtin = m2c.tile([R, LC], bfl, tag="dtin", name="dtin")
                    nc.scalar.activation(dtin[:], ps_xd[0:R, :], AF.Copy)
                    bcl = m2c.tile([64, LC], bfl, tag="bcl", name="bcl")
                    nc.vector.tensor_copy(bcl[32:64, :], ps_xd[32:64, :])
                    nc.sync.dma_start(out=bc_rows[:, ls], in_=bcl[32:64, :])
                    ps_dt = ps2.tile([128, LC], fp32, tag="zxd", name="ps_dt")
                    nc.tensor.matmul(ps_dt[:], sw_dt[:, :], dtin[:],
                                     start=True, stop=True)
                    # softplus(x) = relu(x) + ln(1 + exp(-|x|)) — Abs/Relu
                    # here (tableless), Exp/Ln batched once per stage below
                    nc.scalar.activation(s_ab[:, ts(lc - CPQ * Q, LC)], ps_dt[:],
                                         AF.Abs, bias=sb_dt[:, 0:1])
                    nc.scalar.activation(s_dt[:, ls], ps_dt[:], AF.Relu,
                                         bias=sb_dt[:, 0:1])

                def emit_front(Q):
                    qs = ts(Q, SQ)
                    # phase-1 runs one chunk ahead of phase-2 so the ACT
                    # sigmoids for chunk c+1 aren't queued behind chunk c's
                    # silu/softplus ops
                    pending = None
                    for lc in range(CPQ * Q, CPQ * (Q + 1)):
                        comb = emit_ph1(lc)
                        if pending is not None:
                            emit_ph2(pending[0], pending[1], Q)
                        pending = (lc, comb)
                    emit_ph2(pending[0], pending[1], Q)
                    nc.scalar.activation(s_ex[:], s_ab[:], AF.Exp, scale=-1.0)
                    nc.scalar.activation(s_ab[:], s_ex[:], AF.Ln, bias=1.0)
                    nc.vector.tensor_tensor(s_dt[:, qs], s_dt[:, qs], s_ab[:],
                                            OP.add)
                    # dtu / uD for the quarter
                    nc.vector.tensor_tensor(s_dtu[:, qs], s_dt[:, qs],
                                            s_u[:, 0, qs], OP.mult)
                    nc.vector.tensor_scalar(s_uD[:, qs], s_u[:, 0, qs],
                                            s_d[:, 0:1], None, OP.mult)

                def emit_scan(Q):
                    # scan for stage Q (h carries across stages)
                    qs = ts(Q, SQ)
                    psum_y = psy.tile([128, SQ], fp32, tag="psy", name="psum_y")
                    for piece in range(CPQ):
                        nc.tensor.matmul(psum_y[:, ts(piece, LC)], s_eye[:],
                                         s_uD[:, ts(CPQ * Q + piece, LC)],
                                         start=True, stop=False)
                    for n in range(N):
                        h = spool_h.tile([128, SQ], bfl, tag="h", name="h")
                        a = spool_ab.tile([128, SQ], bfl, tag="a", name="a")
                        bt = spool_ab.tile([128, SQ], bfl, tag="b", name="bt")
                        bbc = spool_bc.tile([128, SQ], bfl, tag="bbc", name="bbc")
                        nc.sync.dma_start(out=bbc[:], in_=bass.AP(
                            tensor=bca.tensor, offset=bca.offset + n * L + Q * SQ,
                            ap=[[0, 128], [1, SQ]]))
                        nc.scalar.activation(a[:], s_dt[:, qs], AF.Exp,
                                             scale=s_a[:, n:n + 1])
                        if n % 2 == 1:
                            nc.gpsimd.tensor_tensor(bt[:], s_dtu[:, qs], bbc[:], OP.mult)
                        else:
                            nc.vector.tensor_tensor(bt[:], s_dtu[:, qs], bbc[:], OP.mult)
                        init = 0.0 if Q == 0 else s_carry[:, n:n + 1]
                        nc.vector.tensor_tensor_scan(h[:], a[:], bt[:], init,
                                                     OP.mult, OP.add)
                        if Q < NQ - 1:
                            nc.vector.tensor_copy(s_carry[:, n:n + 1], h[:, SQ - 1:SQ])
                        cbc = spool_bc.tile([128, SQ], bfl, tag="cbc", name="cbc")
                        nc.sync.dma_start(out=cbc[:], in_=bass.AP(
                            tensor=bca.tensor, offset=bca.offset + (N + n) * L + Q * SQ,
                            ap=[[0, 128], [1, SQ]]))
                        nc.vector.tensor_tensor(h[:], h[:], cbc[:], OP.mult)
                        for piece in range(CPQ):
                            nc.tensor.matmul(psum_y[:, ts(piece, LC)], s_eye[:],
                                             h[:, ts(piece, LC)],
                                             start=False, stop=(n == N - 1))
                    # y8 for the stage + a2a staging (stage covers NQ//... quarters)
                    nc.vector.tensor_tensor(s_y8[:, qs], psum_y[:],
                                            s_siluz[:, qs], OP.mult)
                    for jq in range(Q * (4 // NQ), (Q + 1) * (4 // NQ)):
                        for g in range(2):
                            j = g * 4 + jq
                            nc.sync.dma_start(out=a2a_in[j * 128:(j + 1) * 128, :],
                                              in_=s_y8[:, ts(jq, LQ)])

                # software-pipeline skew: front(Q+1) is emitted before scan(Q)
                # so the in-order PE queue runs a quarter ahead of the DVE scan
                emit_front(0)
                sw_mo8 = wload(w_mo8, 8, Di, "w_mo8", dt=f8)
                sw_out = wload(w_out, 4, C, "w_out")
                sw_dec_g = wload(w_dec_g, 2, Di, "w_dec_g")
                nc.gpsimd.collective_compute(
                    "AllToAll", mybir.AluOpType.bypass,
                    replica_groups=[[0, 1, 2, 3, 4, 5, 6, 7]],
                    ins=[a2a_warm_in[:, :]], outs=[a2a_warm_out[:, :]],
                )
                for Q in range(NQ):
                    if Q + 1 < NQ:
                        emit_front(Q + 1)
                    emit_scan(Q)

            # ---- 8-rank AllToAll (fp8) ----
            nc.gpsimd.collective_compute(
                "AllToAll", mybir.AluOpType.bypass,
                replica_groups=[[0, 1, 2, 3, 4, 5, 6, 7]],
                ins=[a2a_in[:, :]], outs=[a2a_out[:, :]],
            )

            # ---- decoder gate path (fills the AllToAll latency bubble) ----
            ptpool = stack.enter_context(tc.tile_pool(name="pretail", bufs=1))
            s_decf = ptpool.tile([128, 2, LQ], fp32)
            s_decq = ptpool.tile([128, 2, LQ], bfl)
            s_sgate = ptpool.tile([128, 4, LQ], bfl)
            nc.sync.dma_start(out=s_decf[:],
                              in_=dec_f32q.ap().rearrange("(t k) l -> k t l", k=128))
            nc.vector.tensor_copy(s_decq[:], s_decf[:])
            with tc.tile_pool(name="psg", bufs=2, space="PSUM") as psg:
                for lc in range(LQ // LC):
                    ls = ts(lc, LC)
                    for m in range(4):
                        ps_g = psg.tile([128, LC], fp32, tag="ps_g", name="ps_g")
                        for t in range(2):
                            nc.tensor.matmul(ps_g[:], sw_dec_g[:, t, ts(m, 128)],
                                             s_decq[:, t, ls], start=(t == 0), stop=(t == 1))
                        nc.scalar.activation(s_sgate[:, m, ls], ps_g[:], AF.Sigmoid,
                                             bias=sb_dec_g[:, m, :])

            # ---- tail on own L-quarter ----
            with tc.tile_pool(name="tail", bufs=2) as tpool, \
                 tc.tile_pool(name="tail1", bufs=1) as t1pool, \
                 tc.tile_pool(name="ps3", bufs=2, space="PSUM") as ps3, \
                 tc.tile_pool(name="ps3s", bufs=1, space="PSUM") as ps3s:
                s_yall = t1pool.tile([128, 8, LQ], f8)
                a2a_r = a2a_out[:, :].rearrange("(t k) l -> k t l", k=128)
                for pc in range(2):
                    pl = ts(pc, LC)
                    nc.sync.dma_start(out=s_yall[:, :, pl], in_=a2a_r[:, :, pl])
                s_res = t1pool.tile([128, 2, LQ], fp32)
                s_resb = t1pool.tile([128, 2, LQ], bfl)
                s_res2 = t1pool.tile([128, 2, LQ], bfl)
                ones = t1pool.tile([128, 1], bfl)
                nc.vector.memset(ones[:], 1.0)

                s_gated = t1pool.tile([128, 4, LQ], bfl)
                NLQ = LQ // LC
                ps_sum = [ps3s.tile([1, LC], fp32, tag=f"ps_sum{lc}", name=f"ps_sum{lc}")
                          for lc in range(NLQ)]
                ps_sq = [ps3s.tile([1, LC], fp32, tag=f"ps_sq{lc}", name=f"ps_sq{lc}")
                         for lc in range(NLQ)]
                for lc in range(NLQ):
                    ls = ts(lc, LC)
                    for m in range(4):
                        ps_mo = ps3.tile([128, LC], fp32, tag="mm3", name="ps_mo")
                        for t in range(8):
                            nc.tensor.matmul(ps_mo[:], sw_mo8[:, t, ts(m, 128)],
                                             s_yall[:, t, ls], start=(t == 0), stop=(t == 7))
                        spr = tpool.tile([128, LC], bfl, tag="spr", name="spr")
                        nc.scalar.activation(spr[:], ps_mo[:], AF.Identity,
                                             bias=sb_mo[:, m, :], scale=1.0 / (YSC * WSC))
                        nc.vector.tensor_tensor(s_gated[:, m, ls], spr[:],
                                                s_sgate[:, m, ls], OP.mult)
                    for m in range(2):
                        ps_o = ps3.tile([128, LC], fp32, tag="mm3", name="ps_o")
                        for t in range(4):
                            nc.tensor.matmul(ps_o[:], sw_out[:, t, ts(m, 128)],
                                             s_gated[:, t, ls], start=(t == 0), stop=(t == 3))
                        nc.vector.scalar_tensor_tensor(s_res[:, m, ls], ps_o[:],
                                                       sb_out[:, m, :], s_decf[:, m, ls],
                                                       OP.add, OP.add)
                        nc.scalar.activation(s_resb[:, m, ls], s_res[:, m, ls], AF.Copy)
                        nc.scalar.activation(s_res2[:, m, ls], s_res[:, m, ls], AF.Square)
                        nc.tensor.matmul(ps_sum[lc][:], ones[:], s_resb[:, m, ls],
                                         start=(m == 0), stop=(m == 1))
                        nc.tensor.matmul(ps_sq[lc][:], ones[:], s_res2[:, m, ls],
                                         start=(m == 0), stop=(m == 1))

                mu = t1pool.tile([1, LQ], fp32)
                musq = t1pool.tile([1, LQ], fp32)
                var = t1pool.tile([1, LQ], fp32)
                sd = t1pool.tile([1, LQ], fp32)
                rstd = t1pool.tile([1, LQ], fp32)
                for lc in range(NLQ):
                    ls = ts(lc, LC)
                    nc.scalar.activation(mu[:, ls], ps_sum[lc][:], AF.Copy, scale=1.0 / C)
                    nc.scalar.activation(musq[:, ls], mu[:, ls], AF.Square)
                    nc.vector.scalar_tensor_tensor(var[:, ls], ps_sq[lc][:], 1.0 / C,
                                                   musq[:, ls], OP.mult, OP.subtract)
                eps = t1pool.tile([1, 1], fp32)
                nc.vector.memset(eps[:], 1e-5)
                nc.scalar.activation(sd[:], var[:], AF.Sqrt, bias=eps[:, 0:1])
                nc.vector.reciprocal(rstd[:], sd[:])
                mu_bc = t1pool.tile([128, LQ], fp32)
                rs_bc = t1pool.tile([128, LQ], fp32)
                nc.gpsimd.partition_broadcast(mu_bc[:], mu[:])
                nc.gpsimd.partition_broadcast(rs_bc[:], rstd[:])
                for m in range(2):
                    t1 = tpool.tile([128, LQ], fp32, tag="t1", name="t1")
                    nc.vector.tensor_tensor(t1[:], s_res[:, m, :], mu_bc[:], OP.subtract)
                    nc.vector.tensor_tensor(t1[:], t1[:], rs_bc[:], OP.mult)
                    nc.scalar.activation(t1[:], t1[:], AF.Identity,
                                         scale=s_g[:, m, :], bias=s_bln[:, m, :])
                    nc.sync.dma_start(
                        out=res_out.ap().rearrange("(t k) l -> k t l", k=128)[:, m, :],
                        in_=t1[:])

    nc.compile()
    return nc


def _in_maps(inp):
    A = -np.exp(inp["A_log"].astype(np.float32))
    dec_T = inp["decoder_feat"].reshape(B, C, L)
    enc_T = inp["encoder_feat"].reshape(B, C, L)
    dec_T_bf = dec_T.astype(bf16)
    enc_T_bf = enc_T.astype(bf16)

    def col(x):
        return np.ascontiguousarray(np.asarray(x, np.float32).reshape(-1, 1))

    common = {
        "w_dec_x": np.ascontiguousarray(inp["dec_w"][:, :Di].astype(bf16)),
        "w_dec_g": np.ascontiguousarray(inp["dec_w"][:, Di:].astype(bf16)),
        "b_dec_x": col(inp["dec_b"][:Di]),
        "b_dec_g": col(inp["dec_b"][Di:]),
        "w_enc": inp["enc_w"].astype(bf16),
        "b_enc": col(inp["enc_b"]),
        "b_mo": col(inp["m_out_b"]),
        "w_out": inp["out_w"].astype(bf16),
        "b_out": col(inp["out_b"]),
        "g_col": col(inp["ln_g"]),
        "bln_col": col(inp["ln_b"]),
        "eye4k": (np.eye(128, dtype=np.float32) * YSC).astype(bf16),
    }

    in_maps = []
    for c in range(NCORES):
        b, q = c // 4, c % 4
        ds = slice(q * DQ, (q + 1) * DQ)
        perm = np.r_[np.arange(q * DQ, (q + 1) * DQ),
                     np.arange(0, q * DQ), np.arange((q + 1) * DQ, Di)]
        m = dict(common)
        m["dec_bf"] = dec_T_bf[b]
        m["enc_bf"] = enc_T_bf[b]
        m["dec_f32q"] = np.ascontiguousarray(dec_T[b][:, q * LQ:(q + 1) * LQ].astype(np.float32))
        m["w_in_x"] = np.ascontiguousarray(inp["in_w"][:, :Di][:, perm].astype(bf16))
        m["b_in_x"] = col(inp["in_b"][:Di][perm])
        m["w_in_z"] = np.ascontiguousarray(
            inp["in_w"][:, Di + q * DQ:Di + (q + 1) * DQ].astype(bf16))
        m["b_in_z"] = col(inp["in_b"][Di + q * DQ:Di + (q + 1) * DQ])
        m["conv_w4"] = np.ascontiguousarray(inp["conv_w"][perm, 0, :].astype(np.float32))
        m["conv_b"] = col(inp["conv_b"][perm])
        m["w_xp"] = np.ascontiguousarray(inp["x_proj_w"][perm, :].astype(bf16))
        m["w_dt"] = np.ascontiguousarray(inp["dt_w"][:, ds].astype(bf16))
        m["b_dt"] = col(inp["dt_b"][ds])
        wmo8 = np.zeros((2 * Di, Di), np.float32)
        for r in range(8):
            if r // 4 == b:
                rq = r % 4
                wmo8[r * DQ:(r + 1) * DQ] = inp["m_out_w"][rq * DQ:(rq + 1) * DQ]
        m["w_mo8"] = (wmo8 * WSC).astype(f8e4)
        m["a_sl"] = np.ascontiguousarray(A[ds])
        m["d_col"] = col(inp["D_param"][ds])
        in_maps.append(m)
    return in_maps


def kernel(**inputs):
    from concourse.bass_utils import run_bass_kernel_spmd

    inp = {k: np.asarray(v) for k, v in inputs.items()}
    if "nc" not in _cache:
        _cache["nc"] = _build()
    res = run_bass_kernel_spmd(_cache["nc"], _in_maps(inp), list(range(NCORES)))
    out = np.zeros((B, C, L), np.float32)
    for c in range(NCORES):
        b, q = c // 4, c % 4
        out[b][:, q * LQ:(q + 1) * LQ] = res.results[c]["res"]
    return out.reshape(B, C, Hh, Ww)


def run_traced(inp):
    from concourse.bass_utils import run_bass_kernel_spmd

    if "nc" not in _cache:
        _cache["nc"] = _build()
    return run_bass_kernel_spmd(_cache["nc"], _in_maps(inp), list(range(NCORES)),
                                trace=True)


# revision 25
# speedup vs baseline: 1.0852x; 1.0852x over previous
"""CrossMambaFusion Trainium2 kernel — 8-core SPMD via bass/Tile.

Sharding (hardcoded for B=2, C=256, H=W=64, Di=512, N=16, R=32, K=4):
  core c -> batch b = c//4, d_inner slice q = c%4 (128 channels).
  On-device layout is feature-major [features, tokens]; (B,C,H,W) inputs
  reshape to (C, L=4096) with no host transpose.

  SPMD trick 1: xm/u channel order is permuted per core (own d-slice first)
  via host-permuted weights, so "my slice" is always k-tile 0.
  SPMD trick 2: 8-rank AllToAll with duplicated quarter-shards; the m_out
  weight rows belonging to the other batch group are zeroed per core, so the
  K=1024 contraction drops cross-batch contributions (static offsets, SPMD).

v5 structure — the front and the selective scan are software-pipelined per
L-quarter: while the DVE scans quarter Q, the PE/ACT run the projection
front of quarter Q+1 ahead of it (in-order engine queues + Tile semaphores).
  - per quarter: phase1 (dec/enc proj+gate), in_proj, causal conv (DVE STT),
    x_proj, dt softplus (batched Exp/Ln tables), then 16-state scan with
    cross-quarter h carries; ysum accumulates on the PE via 4096*I matmuls
    into PSUM (fp8 pre-scale for free)
  - y ships through the 8-rank AllToAll as fp8e4m3; m_out runs fp8
  - the decoder-gate path fills the AllToAll latency bubble
"""
import numpy as np
import ml_dtypes

bf16 = ml_dtypes.bfloat16
f8e4 = ml_dtypes.float8_e4m3

B, C, Hh, Ww = 2, 256, 64, 64
L = Hh * Ww
Di, N, R, KC = 512, 16, 32, 4
DQ = 128
LQ = L // 4
NCORES = 8
LH = L // 2

YSC = 4096.0   # y fp8 pre-scale (via the eye accumulation matrix)
WSC = 256.0    # m_out weight fp8 pre-scale

_cache = {}


def _build():
    import concourse.bass as bass
    import concourse.mybir as mybir
    import concourse.tile as tile
    from concourse import bacc

    fp32 = mybir.dt.float32
    bfl = mybir.dt.bfloat16
    f8 = mybir.dt.float8e4
    AF = mybir.ActivationFunctionType
    OP = mybir.AluOpType
    ts = bass.ts

    nc = bacc.Bacc("TRN2", target_bir_lowering=False, num_devices=NCORES)

    def din(name, shape, dt=fp32):
        return nc.declare_dram_parameter(name, list(shape), dt, isOutput=False)

    dec_bf = din("dec_bf", (C, L), bfl)
    enc_bf = din("enc_bf", (C, L), bfl)
    dec_f32q = din("dec_f32q", (C, LQ), fp32)
    w_dec_x = din("w_dec_x", (C, Di), bfl)
    w_dec_g = din("w_dec_g", (C, Di), bfl)
    b_dec_x = din("b_dec_x", (Di, 1))
    b_dec_g = din("b_dec_g", (Di, 1))
    w_enc = din("w_enc", (C, Di), bfl)
    b_enc = din("b_enc", (Di, 1))
    w_in_x = din("w_in_x", (Di, Di), bfl)      # columns permuted (own slice first)
    b_in_x = din("b_in_x", (Di, 1))            # permuted
    w_in_z = din("w_in_z", (Di, DQ), bfl)
    b_in_z = din("b_in_z", (DQ, 1))
    conv_w4 = din("conv_w4", (Di, KC))         # permuted rows
    conv_b = din("conv_b", (Di, 1))            # permuted
    w_xp = din("w_xp", (Di, 2 * R), bfl)       # permuted rows
    w_dt = din("w_dt", (R, DQ), bfl)
    b_dt = din("b_dt", (DQ, 1))
    a_sl = din("a_sl", (DQ, N))
    d_col = din("d_col", (DQ, 1))
    w_mo8 = din("w_mo8", (2 * Di, Di), f8)     # rows of other batch group zeroed; x256
    b_mo = din("b_mo", (Di, 1))
    w_out = din("w_out", (Di, C), bfl)
    b_out = din("b_out", (C, 1))
    g_col = din("g_col", (C, 1))
    bln_col = din("bln_col", (C, 1))
    eye4k = din("eye4k", (128, 128), bfl)      # 4096 * I

    res_out = nc.declare_dram_parameter("res", [C, LQ], fp32, isOutput=True)

    LC = 512
    NL = L // LC
    NQ = 2          # pipeline stages (L halves)
    SQ = L // NQ    # tokens per stage
    CPQ = NL // NQ  # 512-col chunks per stage

    with tile.TileContext(nc) as tc:
        import contextlib
        with contextlib.ExitStack() as stack:
            wpool = stack.enter_context(tc.tile_pool(name="weights", bufs=1))
            cpool = stack.enter_context(tc.tile_pool(name="consts", bufs=1))
            dpool = stack.enter_context(tc.tile_pool(name="drambuf", bufs=1, space="DRAM"))

            bc_rows = dpool.tile([2 * N, L], bfl)     # B rows then C rows
            ln_rows = dpool.tile([2, LQ], fp32)       # mu, rstd
            a2a_in = dpool.tile([2 * Di, LQ], f8)
            a2a_out = dpool.tile([2 * Di, LQ], f8)
            a2a_warm_in = dpool.tile([8, 16], f8)
            a2a_warm_out = dpool.tile([8, 16], f8)

            def wload(ap, kt, m, name, dt=bfl):
                t = wpool.tile([128, kt, m], dt, tag=name, name=name)
                nc.sync.dma_start(out=t[:], in_=ap.ap().rearrange("(t k) m -> k t m", k=128))
                return t

            sw_dec_x = wload(w_dec_x, 2, Di, "w_dec_x")
            sw_enc = wload(w_enc, 2, Di, "w_enc")

            def cload(ap, nt, name, cols=1):
                if nt == 1:
                    t = cpool.tile([128, cols], fp32, tag=name, name=name)
                    nc.sync.dma_start(out=t[:], in_=ap.ap())
                else:
                    t = cpool.tile([128, nt, cols], fp32, tag=name, name=name)
                    nc.sync.dma_start(out=t[:], in_=ap.ap().rearrange("(t k) o -> k t o", k=128))
                return t

            sb_dec_x = cload(b_dec_x, 4, "b_dec_x")
            sb_dec_g = cload(b_dec_g, 4, "b_dec_g")
            sb_enc = cload(b_enc, 4, "b_enc")
            sb_in_x = cload(b_in_x, 4, "b_in_x")
            sb_in_z = cload(b_in_z, 1, "b_in_z")
            s_convw = cload(conv_w4, 4, "conv_w4", cols=KC)
            s_convb = cload(conv_b, 4, "conv_b")
            sb_dt = cload(b_dt, 1, "b_dt")
            s_a = cload(a_sl, 1, "a_sl", cols=N)
            s_d = cload(d_col, 1, "d_col")
            sb_mo = cload(b_mo, 4, "b_mo")
            sb_out = cload(b_out, 2, "b_out")
            s_g = cload(g_col, 2, "g_col")
            s_bln = cload(bln_col, 2, "bln_col")

            # persistent tiles
            spool0 = stack.enter_context(tc.tile_pool(name="scanin", bufs=1))
            s_dt = spool0.tile([128, L], bfl)
            s_dtu = spool0.tile([128, L], bfl)
            s_uD = spool0.tile([128, L], bfl)
            s_siluz = spool0.tile([128, L], bfl)
            s_y8 = spool0.tile([128, L], f8)
            s_carry = spool0.tile([128, N], bfl)      # per-n h carries across quarters
            s_ab = spool0.tile([128, L // 2], bfl)    # softplus staging (per stage)
            s_ex = spool0.tile([128, L // 2], bfl)

            sw_in_x = wload(w_in_x, 4, Di, "w_in_x")
            sw_in_z = wload(w_in_z, 4, DQ, "w_in_z")
            sw_xp = wload(w_xp, 4, 2 * R, "w_xp")
            sw_dt = wpool.tile([R, DQ], bfl)
            nc.sync.dma_start(out=sw_dt[:], in_=w_dt.ap())
            s_eye = wpool.tile([128, 128], bfl)
            nc.sync.dma_start(out=s_eye[:], in_=eye4k.ap())

            bca = bc_rows[0:1, 0:1]

            # ---- pipelined front + scan, one L-quarter at a time ----
            with tc.tile_pool(name="ph12", bufs=1) as ppool, \
                 tc.tile_pool(name="ph1c", bufs=3) as f1c, \
                 tc.tile_pool(name="ph2c", bufs=2) as m2c, \
                 tc.tile_pool(name="cmb", bufs=2) as cmbp, \
                 tc.tile_pool(name="scanh", bufs=3) as spool_h, \
                 tc.tile_pool(name="scanab", bufs=2) as spool_ab, \
                 tc.tile_pool(name="scanbc", bufs=2) as spool_bc, \
                 tc.tile_pool(name="ps1", bufs=1, space="PSUM") as ps1, \
                 tc.tile_pool(name="ps2x", bufs=1, space="PSUM") as ps2x, \
                 tc.tile_pool(name="ps2", bufs=1, space="PSUM") as ps2, \
                 tc.tile_pool(name="psy", bufs=1, space="PSUM") as psy:
                s_dec = ppool.tile([128, 2, L], bfl)
                s_enc = ppool.tile([128, 2, L], bfl)
                s_u = ppool.tile([128, 4, L], bfl)
                dec_r = dec_bf.ap().rearrange("(t k) l -> k t l", k=128)
                enc_r = enc_bf.ap().rearrange("(t k) l -> k t l", k=128)
                for pc in range(4):
                    pl = ts(pc, LQ)
                    nc.sync.dma_start(out=s_dec[:, :, pl], in_=dec_r[:, :, pl])
                    nc.sync.dma_start(out=s_enc[:, :, pl], in_=enc_r[:, :, pl])
                prev_xm = [None]

                def emit_ph1(lc):
                    ls = ts(lc, LC)
                    s_comb = cmbp.tile([128, 4, LC], bfl, tag="comb", name="comb")
                    for m in range(4):
                        ps_dx = ps1.tile([128, LC], fp32, tag="ps_dx", name="ps_dx")
                        ps_ep = ps1.tile([128, LC], fp32, tag="ps_ep", name="ps_ep")
                        for t in range(2):
                            nc.tensor.matmul(ps_dx[:], sw_dec_x[:, t, ts(m, 128)],
                                             s_dec[:, t, ls], start=(t == 0), stop=(t == 1))
                        for t in range(2):
                            nc.tensor.matmul(ps_ep[:], sw_enc[:, t, ts(m, 128)],
                                             s_enc[:, t, ls], start=(t == 0), stop=(t == 1))
                        sg = f1c.tile([128, LC], bfl, tag="sg", name="sg")
                        nc.scalar.activation(sg[:], ps_ep[:], AF.Sigmoid,
                                             bias=sb_enc[:, m, :])
                        tm = f1c.tile([128, LC], bfl, tag="tm", name="tm")
                        nc.vector.scalar_tensor_tensor(tm[:], ps_dx[:],
                                                       sb_dec_x[:, m, :], sg[:],
                                                       OP.add, OP.mult)
                        nc.vector.scalar_tensor_tensor(s_comb[:, m, :], ps_ep[:],
                                                       sb_enc[:, m, :], tm[:],
                                                       OP.add, OP.add)
                    return s_comb

                def emit_ph2(lc, s_comb, Q):
                    ls = ts(lc, LC)
                    s_xm = cmbp.tile([128, 4, 3 + LC], bfl, tag="xm", name="s_xm")
                    if prev_xm[0] is None:
                        nc.vector.memset(s_xm[:, :, 0:3], 0.0)
                    else:
                        nc.vector.tensor_copy(s_xm[:, :, 0:3],
                                              prev_xm[0][:, :, LC:LC + 3])
                    for m in range(4):
                        ps_xm = ps2x.tile([128, LC], fp32, tag="ps_xm", name="ps_xm")
                        for t in range(4):
                            nc.tensor.matmul(ps_xm[:], sw_in_x[:, t, ts(m, 128)],
                                             s_comb[:, t, :], start=(t == 0), stop=(t == 3))
                        nc.scalar.activation(s_xm[:, m, 3:3 + LC],
                                             ps_xm[:], AF.Identity, bias=sb_in_x[:, m, :])
                    prev_xm[0] = s_xm
                    ps_z = ps2.tile([128, LC], fp32, tag="zxd", name="ps_z")
                    for t in range(4):
                        nc.tensor.matmul(ps_z[:], sw_in_z[:, t, :], s_comb[:, t, :],
                                         start=(t == 0), stop=(t == 3))
                    nc.scalar.activation(s_siluz[:, ls], ps_z[:], AF.Silu,
                                         bias=sb_in_z[:, 0:1])
                    # causal depthwise conv on DVE (STT chains)
                    for m in range(4):
                        acc = m2c.tile([128, LC], fp32, tag=f"acc{m % 2}", name="acc")
                        nc.vector.tensor_scalar(acc[:], s_xm[:, m, 0:LC],
                                                s_convw[:, m, 0:1], None, OP.mult)
                        for k in range(1, KC):
                            nc.vector.scalar_tensor_tensor(
                                acc[:], s_xm[:, m, k:k + LC],
                                s_convw[:, m, k:k + 1], acc[:], OP.mult, OP.add)
                        nc.scalar.activation(s_u[:, m, ls], acc[:], AF.Silu,
                                             bias=s_convb[:, m, :])
                    # x_proj + B/C spill + dt-linear for this chunk
                    ps_xd = ps2.tile([128, LC], fp32, tag="zxd", name="ps_xd")
                    for t in range(4):
                        nc.tensor.matmul(ps_xd[0:64, :], sw_xp[:, t, :], s_u[:, t, ls],
                                         start=(t == 0), stop=(t == 3))
                    dtin = m2c.tile([R, LC], bfl, tag="dtin", name="dtin")
                    nc.scalar.activation(dtin[:], ps_xd[0:R, :], AF.Copy)
                    bcl = m2c.tile([64, LC], bfl, tag="bcl", name="bcl")
                    nc.vector.tensor_copy(bcl[32:64, :], ps_xd[32:64, :])
                    nc.sync.dma_start(out=bc_rows[:, ls], in_=bcl[32:64, :])
                    ps_dt = ps2.tile([128, LC], fp32, tag="zxd", name="ps_dt")
                    nc.tensor.matmul(ps_dt[:], sw_dt[:, :], dtin[:],
                                     start=True, stop=True)
                    # softplus(x) = relu(x) + ln(1 + exp(-|x|)) — Abs/Relu
                    # here (tableless), Exp/Ln batched once per stage below
                    nc.scalar.activation(s_ab[:, ts(lc - CPQ * Q, LC)], ps_dt[:],
                                         AF.Abs, bias=sb_dt[:, 0:1])
                    nc.scalar.activation(s_dt[:, ls], ps_dt[:], AF.Relu,
                                         bias=sb_dt[:, 0:1])

                def emit_front(Q):
                    qs = ts(Q, SQ)
                    # phase-1 runs one chunk ahead of phase-2 so the ACT
                    # sigmoids for chunk c+1 aren't queued behind chunk c's
                    # silu/softplus ops
                    pending = None
                    for lc in range(CPQ * Q, CPQ * (Q + 1)):
                        comb = emit_ph1(lc)
                        if pending is not None:
                            emit_ph2(pending[0], pending[1], Q)
                        pending = (lc, comb)
                    emit_ph2(pending[0], pending[1], Q)
                    nc.scalar.activation(s_ex[:], s_ab[:], AF.Exp, scale=-1.0)
                    nc.scalar.activation(s_ab[:], s_ex[:], AF.Ln, bias=1.0)
                    nc.vector.tensor_tensor(s_dt[:, qs], s_dt[:, qs], s_ab[:],
                                            OP.add)
                    # dtu / uD for the quarter
                    nc.vector.tensor_tensor(s_dtu[:, qs], s_dt[:, qs],
                                            s_u[:, 0, qs], OP.mult)
                    nc.vector.tensor_scalar(s_uD[:, qs], s_u[:, 0, qs],
                                            s_d[:, 0:1], None, OP.mult)

                def emit_scan(Q):
                    # scan for stage Q (h carries across stages)
                    qs = ts(Q, SQ)
                    psum_y = psy.tile([128, SQ], fp32, tag="psy", name="psum_y")
                    for piece in range(CPQ):
                        nc.tensor.matmul(psum_y[:, ts(piece, LC)], s_eye[:],
                                         s_uD[:, ts(CPQ * Q + piece, LC)],
                                         start=True, stop=False)
                    for n in range(N):
                        h = spool_h.tile([128, SQ], bfl, tag="h", name="h")
                        a = spool_ab.tile([128, SQ], bfl, tag="a", name="a")
                        bt = spool_ab.tile([128, SQ], bfl, tag="b", name="bt")
                        bbc = spool_bc.tile([128, SQ], bfl, tag="bbc", name="bbc")
                        nc.sync.dma_start(out=bbc[:], in_=bass.AP(
                            tensor=bca.tensor, offset=bca.offset + n * L + Q * SQ,
                            ap=[[0, 128], [1, SQ]]))
                        nc.scalar.activation(a[:], s_dt[:, qs], AF.Exp,
                                             scale=s_a[:, n:n + 1])
                        nc.vector.tensor_tensor(bt[:], s_dtu[:, qs], bbc[:], OP.mult)
                        init = 0.0 if Q == 0 else s_carry[:, n:n + 1]
                        nc.vector.tensor_tensor_scan(h[:], a[:], bt[:], init,
                                                     OP.mult, OP.add)
                        if Q < NQ - 1:
                            nc.vector.tensor_copy(s_carry[:, n:n + 1], h[:, SQ - 1:SQ])
                        cbc = spool_bc.tile([128, SQ], bfl, tag="cbc", name="cbc")
                        nc.sync.dma_start(out=cbc[:], in_=bass.AP(
                            tensor=bca.tensor, offset=bca.offset + (N + n) * L + Q * SQ,
                            ap=[[0, 128], [1, SQ]]))
                        nc.vector.tensor_tensor(h[:], h[:], cbc[:], OP.mult)
                        for piece in range(CPQ):
                            nc.tensor.matmul(psum_y[:, ts(piece, LC)], s_eye[:],
                                             h[:, ts(piece, LC)],
                                             start=False, stop=(n == N - 1))
                    # y8 for the stage + a2a staging (stage covers NQ//... quarters)
                    nc.vector.tensor_tensor(s_y8[:, qs], psum_y[:],
                                            s_siluz[:, qs], OP.mult)
                    for jq in range(Q * (4 // NQ), (Q + 1) * (4 // NQ)):
                        for g in range(2):
                            j = g * 4 + jq
                            nc.sync.dma_start(out=a2a_in[j * 128:(j + 1) * 128, :],
                                              in_=s_y8[:, ts(jq, LQ)])

                # software-pipeline skew: front(Q+1) is emitted before scan(Q)
                # so the in-order PE queue runs a quarter ahead of the DVE scan
                emit_front(0)
                sw_mo8 = wload(w_mo8, 8, Di, "w_mo8", dt=f8)
                sw_out = wload(w_out, 4, C, "w_out")
                sw_dec_g = wload(w_dec_g, 2, Di, "w_dec_g")
                nc.gpsimd.collective_compute(
                    "AllToAll", mybir.AluOpType.bypass,
                    replica_groups=[[0, 1, 2, 3, 4, 5, 6, 7]],
                    ins=[a2a_warm_in[:, :]], outs=[a2a_warm_out[:, :]],
                )
                for Q in range(NQ):
                    if Q + 1 < NQ:
                        emit_front(Q + 1)
                    emit_scan(Q)

            # ---- 8-rank AllToAll (fp8) ----
            nc.gpsimd.collective_compute(
                "AllToAll", mybir.AluOpType.bypass,
                replica_groups=[[0, 1, 2, 3, 4, 5, 6, 7]],
                ins=[a2a_in[:, :]], outs=[a2a_out[:, :]],
            )

            # ---- decoder gate path (fills the AllToAll latency bubble) ----
            ptpool = stack.enter_context(tc.tile_pool(name="pretail", bufs=1))
            s_decf = ptpool.tile([128, 2, LQ], fp32)
            s_decq = ptpool.tile([128, 2, LQ], bfl)
            s_sgate = ptpool.tile([128, 4, LQ], bfl)
            nc.sync.dma_start(out=s_decf[:],
                              in_=dec_f32q.ap().rearrange("(t k) l -> k t l", k=128))
            nc.vector.tensor_copy(s_decq[:], s_decf[:])
            with tc.tile_pool(name="psg", bufs=2, space="PSUM") as psg:
                for lc in range(LQ // LC):
                    ls = ts(lc, LC)
                    for m in range(4):
                        ps_g = psg.tile([128, LC], fp32, tag="ps_g", name="ps_g")
                        for t in range(2):
                            nc.tensor.matmul(ps_g[:], sw_dec_g[:, t, ts(m, 128)],
                                             s_decq[:, t, ls], start=(t == 0), stop=(t == 1))
                        nc.scalar.activation(s_sgate[:, m, ls], ps_g[:], AF.Sigmoid,
                                             bias=sb_dec_g[:, m, :])

            # ---- tail on own L-quarter ----
            with tc.tile_pool(name="tail", bufs=2) as tpool, \
                 tc.tile_pool(name="tail1", bufs=1) as t1pool, \
                 tc.tile_pool(name="ps3", bufs=2, space="PSUM") as ps3, \
                 tc.tile_pool(name="ps3s", bufs=1, space="PSUM") as ps3s:
                s_yall = t1pool.tile([128, 8, LQ], f8)
                a2a_r = a2a_out[:, :].rearrange("(t k) l -> k t l", k=128)
                for pc in range(2):
                    pl = ts(pc, LC)
                    nc.sync.dma_start(out=s_yall[:, :, pl], in_=a2a_r[:, :, pl])
                s_res = t1pool.tile([128, 2, LQ], fp32)
                s_resb = t1pool.tile([128, 2, LQ], bfl)
                s_res2 = t1pool.tile([128, 2, LQ], bfl)
                ones = t1pool.tile([128, 1], bfl)
                nc.vector.memset(ones[:], 1.0)

                s_gated = t1pool.tile([128, 4, LQ], bfl)
                NLQ = LQ // LC
                ps_sum = [ps3s.tile([1, LC], fp32, tag=f"ps_sum{lc}", name=f"ps_sum{lc}")
                          for lc in range(NLQ)]
                ps_sq = [ps3s.tile([1, LC], fp32, tag=f"ps_sq{lc}", name=f"ps_sq{lc}")
                         for lc in range(NLQ)]
                for lc in range(NLQ):
                    ls = ts(lc, LC)
                    for m in range(4):
                        ps_mo = ps3.tile([128, LC], fp32, tag="mm3", name="ps_mo")
                        for t in range(8):
                            nc.tensor.matmul(ps_mo[:], sw_mo8[:, t, ts(m, 128)],
                                             s_yall[:, t, ls], start=(t == 0), stop=(t == 7))
                        spr = tpool.tile([128, LC], bfl, tag="spr", name="spr")
                        nc.scalar.activation(spr[:], ps_mo[:], AF.Identity,
                                             bias=sb_mo[:, m, :], scale=1.0 / (YSC * WSC))
                        nc.vector.tensor_tensor(s_gated[:, m, ls], spr[:],
                                                s_sgate[:, m, ls], OP.mult)
                    for m in range(2):
                        ps_o = ps3.tile([128, LC], fp32, tag="mm3", name="ps_o")
                        for t in range(4):
                            nc.tensor.matmul(ps_o[:], sw_out[:, t, ts(m, 128)],
                                             s_gated[:, t, ls], start=(t == 0), stop=(t == 3))
                        nc.vector.scalar_tensor_tensor(s_res[:, m, ls], ps_o[:],
                                                       sb_out[:, m, :], s_decf[:, m, ls],
                                                       OP.add, OP.add)
                        nc.scalar.activation(s_resb[:, m, ls], s_res[:, m, ls], AF.Copy)
                        nc.scalar.activation(s_res2[:, m, ls], s_res[:, m, ls], AF.Square)
                        nc.tensor.matmul(ps_sum[lc][:], ones[:], s_resb[:, m, ls],
                                         start=(m == 0), stop=(m == 1))
                        nc.tensor.matmul(ps_sq[lc][:], ones[:], s_res2[:, m, ls],
                                         start=(m == 0), stop=(m == 1))

                mu = t1pool.tile([1, LQ], fp32)
                musq = t1pool.tile([1, LQ], fp32)
                var = t1pool.tile([1, LQ], fp32)
                sd = t1pool.tile([1, LQ], fp32)
                rstd = t1pool.tile([1, LQ], fp32)
                for lc in range(NLQ):
                    ls = ts(lc, LC)
                    nc.scalar.activation(mu[:, ls], ps_sum[lc][:], AF.Copy, scale=1.0 / C)
                    nc.scalar.activation(musq[:, ls], mu[:, ls], AF.Square)
                    nc.vector.scalar_tensor_tensor(var[:, ls], ps_sq[lc][:], 1.0 / C,
                                                   musq[:, ls], OP.mult, OP.subtract)
                eps = t1pool.tile([1, 1], fp32)
                nc.vector.memset(eps[:], 1e-5)
                nc.scalar.activation(sd[:], var[:], AF.Sqrt, bias=eps[:, 0:1])
                nc.vector.reciprocal(rstd[:], sd[:])
                mu_bc = t1pool.tile([128, LQ], fp32)
                rs_bc = t1pool.tile([128, LQ], fp32)
                nc.gpsimd.partition_broadcast(mu_bc[:], mu[:])
                nc.gpsimd.partition_broadcast(rs_bc[:], rstd[:])
                for m in range(2):
                    t1 = tpool.tile([128, LQ], fp32, tag="t1", name="t1")
                    nc.vector.tensor_tensor(t1[:], s_res[:, m, :], mu_bc[:], OP.subtract)
                    nc.vector.tensor_tensor(t1[:], t1[:], rs_bc[:], OP.mult)
                    nc.scalar.activation(t1[:], t1[:], AF.Identity,
                                         scale=s_g[:, m, :], bias=s_bln[:, m, :])
                    nc.sync.dma_start(
                        out=res_out.ap().rearrange("(t k) l -> k t l", k=128)[:, m, :],
                        in_=t1[:])

    nc.compile()
    return nc


def _in_maps(inp):
    A = -np.exp(inp["A_log"].astype(np.float32))
    dec_T = inp["decoder_feat"].reshape(B, C, L)
    enc_T = inp["encoder_feat"].reshape(B, C, L)
    dec_T_bf = dec_T.astype(bf16)
    enc_T_bf = enc_T.astype(bf16)

    def col(x):
        return np.ascontiguousarray(np.asarray(x, np.float32).reshape(-1, 1))

    common = {
        "w_dec_x": np.ascontiguousarray(inp["dec_w"][:, :Di].astype(bf16)),
        "w_dec_g": np.ascontiguousarray(inp["dec_w"][:, Di:].astype(bf16)),
        "b_dec_x": col(inp["dec_b"][:Di]),
        "b_dec_g": col(inp["dec_b"][Di:]),
        "w_enc": inp["enc_w"].astype(bf16),
        "b_enc": col(inp["enc_b"]),
        "b_mo": col(inp["m_out_b"]),
        "w_out": inp["out_w"].astype(bf16),
        "b_out": col(inp["out_b"]),
        "g_col": col(inp["ln_g"]),
        "bln_col": col(inp["ln_b"]),
        "eye4k": (np.eye(128, dtype=np.float32) * YSC).astype(bf16),
    }

    in_maps = []
    for c in range(NCORES):
        b, q = c // 4, c % 4
        ds = slice(q * DQ, (q + 1) * DQ)
        perm = np.r_[np.arange(q * DQ, (q + 1) * DQ),
                     np.arange(0, q * DQ), np.arange((q + 1) * DQ, Di)]
        m = dict(common)
        m["dec_bf"] = dec_T_bf[b]
        m["enc_bf"] = enc_T_bf[b]
        m["dec_f32q"] = np.ascontiguousarray(dec_T[b][:, q * LQ:(q + 1) * LQ].astype(np.float32))
        m["w_in_x"] = np.ascontiguousarray(inp["in_w"][:, :Di][:, perm].astype(bf16))
        m["b_in_x"] = col(inp["in_b"][:Di][perm])
        m["w_in_z"] = np.ascontiguousarray(
            inp["in_w"][:, Di + q * DQ:Di + (q + 1) * DQ].astype(bf16))
        m["b_in_z"] = col(inp["in_b"][Di + q * DQ:Di + (q + 1) * DQ])
        m["conv_w4"] = np.ascontiguousarray(inp["conv_w"][perm, 0, :].astype(np.float32))
        m["conv_b"] = col(inp["conv_b"][perm])
        m["w_xp"] = np.ascontiguousarray(inp["x_proj_w"][perm, :].astype(bf16))
        m["w_dt"] = np.ascontiguousarray(inp["dt_w"][:, ds].astype(bf16))
        m["b_dt"] = col(inp["dt_b"][ds])
        wmo8 = np.zeros((2 * Di, Di), np.float32)
        for r in range(8):
            if r // 4 == b:
                rq = r % 4
                wmo8[r * DQ:(r + 1) * DQ] = inp["m_out_w"][rq * DQ:(rq + 1) * DQ]
        m["w_mo8"] = (wmo8 * WSC).astype(f8e4)
        m["a_sl"] = np.ascontiguousarray(A[ds])
        m["d_col"] = col(inp["D_param"][ds])
        in_maps.append(m)
    return in_maps


def kernel(**inputs):
    from concourse.bass_utils import run_bass_kernel_spmd

    inp = {k: np.asarray(v) for k, v in inputs.items()}
    if "nc" not in _cache:
        _cache["nc"] = _build()
    res = run_bass_kernel_spmd(_cache["nc"], _in_maps(inp), list(range(NCORES)))
    out = np.zeros((B, C, L), np.float32)
    for c in range(NCORES):
        b, q = c // 4, c % 4
        out[b][:, q * LQ:(q + 1) * LQ] = res.results[c]["res"]
    return out.reshape(B, C, Hh, Ww)


def run_traced(inp):
    from concourse.bass_utils import run_bass_kernel_spmd

    if "nc" not in _cache:
        _cache["nc"] = _build()
    return run_bass_kernel_spmd(_cache["nc"], _in_maps(inp), list(range(NCORES)),
                                trace=True)
